# revision 11
# baseline (speedup 1.0000x reference)
"""Trainium2 Bass kernel for FlowNet/stereo-style horizontal correlation.

Reference semantics (per batch sample b):
    x: [2C, H, W] fp32, f1 = x[:C], f2 = x[C:]
    out[d, h, w] = (1/C) * sum_c f1[c, h, w] * f2[c, h, w - d]   (zero-padded)
with C = 64, D = max_disparity = 64, H = 256, W = 512, B = 4.

Strategy (8 NeuronCores):
  Shard batch (4) x H-halves (2) -> 8 shards of [128c2, 128h, 512w].

  On-device, per (h, t) with t in {0, 64, ..., 448}:
    one TensorE matmul: stationary lhsT = f2 window [c=64, 128 cols]
    covering w' in [t-63, t+64] (zero-padded at edges), moving rhs =
    f1 [c=64, 64 cols] covering w in [t, t+64).  PSUM cell (m, n) =
    sum_c f2[c, t-63+m] * f1[c, t+n] = unnormalized out[d=63+n-m, w=t+n].

  The [128, 64] rectangle is copied (DVE/ACT) into a staging tile and
  DMAed to a DRAM scratch laid out in "band lines": line s~ = t + m,
  slot j^ = 63 + n - m, i.e. flat q = (t+m)*(PITCH*HC) + (63+n-m)*HC + hh.
  Per-partition DRAM offsets are affine in (m, t, n, hh) -> a single
  legal strided DMA with HC-elem-granular contiguous runs of 64*HC
  elems.  Cells with d outside [0, 64) provably land in pad slots
  [64, 128) of some line and are never read.  Host assembles:
  out[d, h, w] = scr[w-d+63, d, h] * (1/C).

  Inputs are packed on the host so each per-chunk DMA covers all 128
  SBUF partitions: partition p = 64*(h&1) + c, free = (h//2, w).
"""

import sys

sys.path.insert(0, "/opt/trn_rl_repo")

import numpy as np

import concourse.bass as bass
import concourse.mybir as mybir
import concourse.tile as tile
from concourse import bacc, bass_utils

# problem constants (hardcoded per contract)
B = 4
C = 64
D = 64
H = 256
W = 512
NCORES = 8
HS = H // 2          # 128 rows per core
HC = 16              # h-chunk size
HP = HC // 2         # h-pairs per chunk (h-parity packed on partitions)
NCHUNK = HS // HC    # 8
TSTEP = 64
NT = W // TSTEP      # 8
PITCH = 128          # j^ pitch (valid slots [0, 64), pad [64, 128))
LINES = W + D        # 576 band lines (s~ = t + m in [0, 575])
WPAD = 584           # f2 padded row: cols [0,64)=0, [64,576)=data, 576.. pad

DT_IN = mybir.dt.float16
DT_OUT = mybir.dt.float16
NP_IN = np.float16
NP_OUT = np.float16


def _corr_kernel(tc, f1_ap, f2_ap, scr_ap):
    nc = tc.nc
    scr_t = scr_ap.tensor
    with (
        tc.tile_pool(name="io", bufs=2) as iopool,
        tc.tile_pool(name="stage", bufs=2) as stpool,
        tc.tile_pool(name="ps", bufs=6, space="PSUM") as pspool,
    ):
        for cc in range(NCHUNK):
            j0 = cc * HP
            f1 = iopool.tile([128, HP, W], DT_IN, tag="f1")
            f2 = iopool.tile([128, HP, WPAD], DT_IN, tag="f2")
            nc.sync.dma_start(f1[:, :, :], f1_ap[:, j0 : j0 + HP, :])
            nc.sync.dma_start(f2[:, :, :], f2_ap[:, j0 : j0 + HP, :])

            st = stpool.tile([128, NT * TSTEP * HC], DT_OUT, tag="st")
            st4 = st.rearrange("p (t n h) -> p t n h", t=NT, n=TSTEP, h=HC)
            for hh in range(HC):
                j, par = hh // 2, hh % 2
                p0 = C * par
                for t4 in range(NT // 4):
                    pt = pspool.tile([128, 4 * TSTEP], mybir.dt.float32, tag="pt")
                    for tq in range(4):
                        t = (t4 * 4 + tq) * TSTEP
                        # lhsT: f2 cols [t-63, t+65) -> padded cols [t+1, t+129)
                        nc.tensor.matmul(
                            pt[:, tq * TSTEP : (tq + 1) * TSTEP],
                            f2[p0 : p0 + C, j, t + 1 : t + 129],
                            f1[p0 : p0 + C, j, t : t + TSTEP],
                        )
                    dst = st4[:, t4 * 4 : (t4 + 1) * 4, :, hh : hh + 1]
                    src = pt.rearrange("p (t n o) -> p t n o", t=4, n=TSTEP, o=1)
                    if (hh * 2 + t4) % 3 == 2:
                        nc.scalar.copy(dst, src)
                    else:
                        nc.vector.tensor_copy(dst, src)

            # staging tile -> DRAM scratch (band-line layout), one DMA
            dram_ap = bass.AP(
                tensor=scr_t,
                offset=cc * LINES * PITCH * HC + 63 * HC,
                ap=[
                    [(PITCH - 1) * HC, 128],   # m (partition)
                    [TSTEP * PITCH * HC, NT],  # t^
                    [HC, TSTEP],               # n
                    [1, HC],                   # hh
                ],
            )
            nc.sync.dma_start(dram_ap, st4[:, :, :, :])


def _build():
    nc = bacc.Bacc("TRN2", target_bir_lowering=False, debug=False)
    f1s = nc.dram_tensor("f1s", [128, HS // 2, W], DT_IN, kind="ExternalInput")
    f2s = nc.dram_tensor("f2s", [128, HS // 2, WPAD], DT_IN, kind="ExternalInput")
    scr = nc.dram_tensor(
        "scr", [NCHUNK, LINES, PITCH, HC], DT_OUT, kind="ExternalOutput"
    )
    with tile.TileContext(nc) as tc:
        _corr_kernel(tc, f1s.ap(), f2s.ap(), scr.ap())
    nc.compile()
    return nc


def _run_on_hw(in_maps, trace=False, **kw):
    nc = _build()
    return bass_utils.run_bass_kernel_spmd(
        nc, in_maps, core_ids=list(range(NCORES)), trace=trace, **kw
    )


def _assemble(scr_cores):
    """scr_cores: list of 8 arrays [NCHUNK, LINES, PITCH, HC] -> [B, D, H, W]."""
    out = np.empty((B, D, H, W), dtype=np.float32)
    for core in range(NCORES):
        b, half = core // 2, core % 2
        scr = scr_cores[core].astype(np.float32, copy=False)
        # out[d, cc*HC+hh, w] = scr[cc, w - d + 63, d, hh] / C
        for d in range(D):
            sl = scr[:, 63 - d : 63 - d + W, d, :]  # [NCHUNK, W, HC]
            out[b, d, half * HS : (half + 1) * HS, :] = (
                sl.transpose(0, 2, 1).reshape(HS, W)
            )
    out *= 1.0 / C
    return out


def _make_in_maps(x):
    x = np.asarray(x)
    assert x.shape == (B, 2 * C, H, W), x.shape
    in_maps = []
    for core in range(NCORES):
        b, half = core // 2, core % 2
        sh = slice(half * HS, (half + 1) * HS)
        f1 = np.asarray(x[b, :C, sh, :], dtype=NP_IN)  # [C, HS, W]
        f2 = np.asarray(x[b, C:, sh, :], dtype=NP_IN)
        # pack: arr[64*par + c, j, w] = f[c, 2j + par, w]
        f1p = np.empty((128, HS // 2, W), dtype=NP_IN)
        f1p[:C] = f1[:, 0::2, :]
        f1p[C:] = f1[:, 1::2, :]
        f2p = np.zeros((128, HS // 2, WPAD), dtype=NP_IN)
        f2p[:C, :, D : D + W] = f2[:, 0::2, :]
        f2p[C:, :, D : D + W] = f2[:, 1::2, :]
        in_maps.append({"f1s": f1p, "f2s": f2p})
    return in_maps


def kernel(x, max_disparity):
    assert int(max_disparity) == D
    res = _run_on_hw(_make_in_maps(x))
    scr_cores = [res.results[core]["scr"] for core in range(NCORES)]
    return _assemble(scr_cores)


# revision 12
# speedup vs baseline: 1.0038x; 1.0038x over previous
"""Trainium2 Bass kernel for FlowNet/stereo-style horizontal correlation.

Reference semantics (per batch sample b):
    x: [2C, H, W] fp32, f1 = x[:C], f2 = x[C:]
    out[d, h, w] = (1/C) * sum_c f1[c, h, w] * f2[c, h, w - d]   (zero-padded)
with C = 64, D = max_disparity = 64, H = 256, W = 512, B = 4.

Strategy (8 NeuronCores):
  Shard batch (4) x H-halves (2) -> 8 shards of [128c2, 128h, 512w].

  On-device, per (h, t) with t in {0, 64, ..., 448}:
    one TensorE matmul: stationary lhsT = f2 window [c=64, 128 cols]
    covering w' in [t-63, t+64] (zero-padded at edges), moving rhs =
    f1 [c=64, 64 cols] covering w in [t, t+64).  PSUM cell (m, n) =
    sum_c f2[c, t-63+m] * f1[c, t+n] = unnormalized out[d=63+n-m, w=t+n].

  The [128, 64] rectangle is copied (DVE/ACT) into a staging tile and
  DMAed to a DRAM scratch laid out in "band lines": line s~ = t + m,
  slot j^ = 63 + n - m, i.e. flat q = (t+m)*(PITCH*HC) + (63+n-m)*HC + hh.
  Per-partition DRAM offsets are affine in (m, t, n, hh) -> a single
  legal strided DMA with HC-elem-granular contiguous runs of 64*HC
  elems.  Cells with d outside [0, 64) provably land in pad slots
  [64, 128) of some line and are never read.  Host assembles:
  out[d, h, w] = scr[w-d+63, d, h] * (1/C).

  Inputs are packed on the host so each per-chunk DMA covers all 128
  SBUF partitions: partition p = 64*(h&1) + c, free = (h//2, w).
"""

import sys

sys.path.insert(0, "/opt/trn_rl_repo")

import numpy as np

import concourse.bass as bass
import concourse.mybir as mybir
import concourse.tile as tile
from concourse import bacc, bass_utils

# problem constants (hardcoded per contract)
B = 4
C = 64
D = 64
H = 256
W = 512
NCORES = 8
HS = H // 2          # 128 rows per core
HC = 16              # h-chunk size
HP = HC // 2         # h-pairs per chunk (h-parity packed on partitions)
NCHUNK = HS // HC    # 8
TSTEP = 64
NT = W // TSTEP      # 8
PITCH = 128          # j^ pitch (valid slots [0, 64), pad [64, 128))
LINES = W + D        # 576 band lines (s~ = t + m in [0, 575])
WPAD = 584           # f2 padded row: cols [0,64)=0, [64,576)=data, 576.. pad

DT_IN = mybir.dt.float16
DT_OUT = mybir.dt.float16
NP_IN = np.float16
NP_OUT = np.float16


def _corr_kernel(tc, f1_ap, f2_ap, scr_ap):
    nc = tc.nc
    scr_t = scr_ap.tensor
    with (
        tc.tile_pool(name="io", bufs=2) as iopool,
        tc.tile_pool(name="stage", bufs=2) as stpool,
        tc.tile_pool(name="ps", bufs=6, space="PSUM") as pspool,
    ):
        for cc in range(NCHUNK):
            j0 = cc * HP
            f1 = iopool.tile([128, HP, W], DT_IN, tag="f1")
            f2 = iopool.tile([128, HP, WPAD], DT_IN, tag="f2")
            nc.sync.dma_start(f1[:, :, :], f1_ap[:, j0 : j0 + HP, :])
            nc.sync.dma_start(f2[:, :, :], f2_ap[:, j0 : j0 + HP, :])

            st = stpool.tile([128, NT * TSTEP * HC], DT_OUT, tag="st")
            st4 = st.rearrange("p (t n h) -> p t n h", t=NT, n=TSTEP, h=HC)
            for hh in range(HC):
                j, par = hh // 2, hh % 2
                p0 = C * par
                for t4 in range(NT // 4):
                    pt = pspool.tile([128, 4 * TSTEP], mybir.dt.float32, tag="pt")
                    for tq in range(4):
                        t = (t4 * 4 + tq) * TSTEP
                        # lhsT: f2 cols [t-63, t+65) -> padded cols [t+1, t+129)
                        nc.tensor.matmul(
                            pt[:, tq * TSTEP : (tq + 1) * TSTEP],
                            f2[p0 : p0 + C, j, t + 1 : t + 129],
                            f1[p0 : p0 + C, j, t : t + TSTEP],
                        )
                    # squeeze the length-1 h dim so the 64-elem n dim is the
                    # innermost AP dim (1-elem inner loops are ~5x slower)
                    dst = st4[:, t4 * 4 : (t4 + 1) * 4, :, hh : hh + 1].squeeze(3)
                    src = pt.rearrange("p (t n) -> p t n", t=4, n=TSTEP)
                    if (hh * 2 + t4) % 3 == 2:
                        nc.scalar.copy(dst, src)
                    else:
                        nc.vector.tensor_copy(dst, src)

            # staging tile -> DRAM scratch (band-line layout), one DMA
            dram_ap = bass.AP(
                tensor=scr_t,
                offset=cc * LINES * PITCH * HC + 63 * HC,
                ap=[
                    [(PITCH - 1) * HC, 128],   # m (partition)
                    [TSTEP * PITCH * HC, NT],  # t^
                    [HC, TSTEP],               # n
                    [1, HC],                   # hh
                ],
            )
            nc.sync.dma_start(dram_ap, st4[:, :, :, :])


def _build():
    nc = bacc.Bacc("TRN2", target_bir_lowering=False, debug=False)
    f1s = nc.dram_tensor("f1s", [128, HS // 2, W], DT_IN, kind="ExternalInput")
    f2s = nc.dram_tensor("f2s", [128, HS // 2, WPAD], DT_IN, kind="ExternalInput")
    scr = nc.dram_tensor(
        "scr", [NCHUNK, LINES, PITCH, HC], DT_OUT, kind="ExternalOutput"
    )
    with tile.TileContext(nc) as tc:
        _corr_kernel(tc, f1s.ap(), f2s.ap(), scr.ap())
    nc.compile()
    return nc


def _run_on_hw(in_maps, trace=False, **kw):
    nc = _build()
    return bass_utils.run_bass_kernel_spmd(
        nc, in_maps, core_ids=list(range(NCORES)), trace=trace, **kw
    )


def _assemble(scr_cores):
    """scr_cores: list of 8 arrays [NCHUNK, LINES, PITCH, HC] -> [B, D, H, W]."""
    out = np.empty((B, D, H, W), dtype=np.float32)
    for core in range(NCORES):
        b, half = core // 2, core % 2
        scr = scr_cores[core].astype(np.float32, copy=False)
        # out[d, cc*HC+hh, w] = scr[cc, w - d + 63, d, hh] / C
        for d in range(D):
            sl = scr[:, 63 - d : 63 - d + W, d, :]  # [NCHUNK, W, HC]
            out[b, d, half * HS : (half + 1) * HS, :] = (
                sl.transpose(0, 2, 1).reshape(HS, W)
            )
    out *= 1.0 / C
    return out


def _make_in_maps(x):
    x = np.asarray(x)
    assert x.shape == (B, 2 * C, H, W), x.shape
    in_maps = []
    for core in range(NCORES):
        b, half = core // 2, core % 2
        sh = slice(half * HS, (half + 1) * HS)
        f1 = np.asarray(x[b, :C, sh, :], dtype=NP_IN)  # [C, HS, W]
        f2 = np.asarray(x[b, C:, sh, :], dtype=NP_IN)
        # pack: arr[64*par + c, j, w] = f[c, 2j + par, w]
        f1p = np.empty((128, HS // 2, W), dtype=NP_IN)
        f1p[:C] = f1[:, 0::2, :]
        f1p[C:] = f1[:, 1::2, :]
        f2p = np.zeros((128, HS // 2, WPAD), dtype=NP_IN)
        f2p[:C, :, D : D + W] = f2[:, 0::2, :]
        f2p[C:, :, D : D + W] = f2[:, 1::2, :]
        in_maps.append({"f1s": f1p, "f2s": f2p})
    return in_maps


def kernel(x, max_disparity):
    assert int(max_disparity) == D
    res = _run_on_hw(_make_in_maps(x))
    scr_cores = [res.results[core]["scr"] for core in range(NCORES)]
    return _assemble(scr_cores)


# revision 18
# speedup vs baseline: 1.4874x; 1.4817x over previous
"""Trainium2 Bass kernel for FlowNet/stereo-style horizontal correlation.

Reference semantics (per batch sample b):
    x: [2C, H, W] fp32, f1 = x[:C], f2 = x[C:]
    out[d, h, w] = (1/C) * sum_c f1[c, h, w] * f2[c, h, w - d]   (zero-padded)
with C = 64, D = max_disparity = 64, H = 256, W = 512, B = 4.

Strategy (8 NeuronCores):
  Shard batch (4) x H-halves (2) -> 8 shards of [128c2, 128h, 512w].

  On-device, per (h, t) with t in {0, 64, ..., 448}:
    one TensorE matmul: stationary lhsT = f2 window [c=64, 128 cols]
    covering w' in [t-63, t+64] (zero-padded at edges), moving rhs =
    f1 [c=64, 64 cols] covering w in [t, t+64).  PSUM cell (m, n) =
    sum_c f2[c, t-63+m] * f1[c, t+n] = unnormalized out[d=63+n-m, w=t+n].

  The [128, 64] rectangle is copied (DVE/ACT) into a staging tile and
  DMAed to a DRAM scratch laid out in "band lines": line s~ = t + m,
  slot j^ = 63 + n - m, i.e. flat q = (t+m)*(PITCH*HC) + (63+n-m)*HC + hh.
  Per-partition DRAM offsets are affine in (m, t, n, hh) -> a single
  legal strided DMA with HC-elem-granular contiguous runs of 64*HC
  elems.  Cells with d outside [0, 64) provably land in pad slots
  [64, 128) of some line and are never read.  Host assembles:
  out[d, h, w] = scr[w-d+63, d, h] * (1/C).

  Inputs are packed on the host so each per-chunk DMA covers all 128
  SBUF partitions: partition p = 64*(h&1) + c, free = (h//2, w).
"""

import os
import sys

sys.path.insert(0, "/opt/trn_rl_repo")

import numpy as np

import concourse.bass as bass
import concourse.mybir as mybir
import concourse.tile as tile
from concourse import bacc, bass_utils

# problem constants (hardcoded per contract)
B = 4
C = 64
D = 64
H = 256
W = 512
NCORES = 8
HS = H // 2          # 128 rows per core
HC = 16              # h-chunk size
HP = HC // 2         # h-pairs per chunk (h-parity packed on partitions)
NCHUNK = HS // HC    # 8
TSTEP = 64
NT = W // TSTEP      # 8
PITCH = 128          # j^ pitch (valid slots [0, 64), pad [64, 128))
LINES = W + D        # 576 band lines (s~ = t + m in [0, 575])
WPAD = 584           # f2 padded row: cols [0,64)=0, [64,576)=data, 576.. pad

DT_IN = mybir.dt.float16
DT_OUT = mybir.dt.float16
NP_IN = np.float16
NP_OUT = np.float16


def _corr_kernel(tc, f1_ap, f2_ap, scr_ap):
    nc = tc.nc
    scr_t = scr_ap.tensor
    with (
        tc.tile_pool(name="io", bufs=2) as iopool,
        tc.tile_pool(name="stage", bufs=2) as stpool,
        tc.tile_pool(name="ps", bufs=int(os.environ.get("K_PS_BUFS", "6")), space="PSUM") as pspool,
    ):
        for cc in range(NCHUNK):
            j0 = cc * HP
            f1 = iopool.tile([128, HP, W], DT_IN, tag="f1")
            f2 = iopool.tile([128, HP, WPAD], DT_IN, tag="f2")
            nc.sync.dma_start(f1[:, :, :], f1_ap[:, j0 : j0 + HP, :])
            nc.sync.dma_start(f2[:, :, :], f2_ap[:, j0 : j0 + HP, :])

            st = stpool.tile([128, NT * TSTEP * HC], DT_OUT, tag="st")
            st4 = st.rearrange("p (t n h) -> p t n h", t=NT, n=TSTEP, h=HC)
            ci = 0
            for oct_ in range(HC // 8):
                for th in range(NT):
                    t = th * TSTEP
                    # 8 matmuls (one per h in the octet) write one PSUM bank
                    # h-interleaved via a strided output AP, so the staging
                    # copy below reads contiguously and writes 8-elem runs.
                    pt = pspool.tile([128, TSTEP, 8], mybir.dt.float32, tag="pt")
                    for hj in range(8):
                        # octet = 8 rows of one parity: par fixed per octet so
                        # the base partition never flips between the strided
                        # matmuls of a group (flipping mid-group crashes HW).
                        j, par = hj, oct_
                        p0 = C * par
                        # lhsT: f2 cols [t-63, t+65) -> padded cols [t+1, t+129)
                        nc.tensor.matmul(
                            pt[:, :, hj],
                            f2[p0 : p0 + C, j, t + 1 : t + 129],
                            f1[p0 : p0 + C, j, t : t + TSTEP],
                        )
                    dst = st4[:, th, :, oct_ * 8 : (oct_ + 1) * 8]
                    act_mod = int(os.environ.get("K_ACT_MOD", "3"))
                    if act_mod and ci % act_mod == act_mod - 1:
                        nc.scalar.copy(dst, pt[:, :, :])
                    else:
                        nc.vector.tensor_copy(dst, pt[:, :, :])
                    ci += 1

            # staging tile -> DRAM scratch (band-line layout), one DMA
            dram_ap = bass.AP(
                tensor=scr_t,
                offset=cc * LINES * PITCH * HC + 63 * HC,
                ap=[
                    [(PITCH - 1) * HC, 128],   # m (partition)
                    [TSTEP * PITCH * HC, NT],  # t^
                    [HC, TSTEP],               # n
                    [1, HC],                   # hh
                ],
            )
            nc.sync.dma_start(dram_ap, st4[:, :, :, :])


def _build():
    nc = bacc.Bacc("TRN2", target_bir_lowering=False, debug=False)
    f1s = nc.dram_tensor("f1s", [128, HS // 2, W], DT_IN, kind="ExternalInput")
    f2s = nc.dram_tensor("f2s", [128, HS // 2, WPAD], DT_IN, kind="ExternalInput")
    scr = nc.dram_tensor(
        "scr", [NCHUNK, LINES, PITCH, HC], DT_OUT, kind="ExternalOutput"
    )
    with tile.TileContext(nc) as tc:
        _corr_kernel(tc, f1s.ap(), f2s.ap(), scr.ap())
    nc.compile()
    return nc


def _run_on_hw(in_maps, trace=False, **kw):
    nc = _build()
    return bass_utils.run_bass_kernel_spmd(
        nc, in_maps, core_ids=list(range(NCORES)), trace=trace, **kw
    )


def _assemble(scr_cores):
    """scr_cores: list of 8 arrays [NCHUNK, LINES, PITCH, HC] -> [B, D, H, W]."""
    out = np.empty((B, D, H, W), dtype=np.float32)
    for core in range(NCORES):
        b, half = core // 2, core % 2
        scr = scr_cores[core].astype(np.float32, copy=False)
        # staging slot s holds h = 2*(s % 8) + (s // 8) within the chunk
        perm = [(h % 2) * 8 + h // 2 for h in range(HC)]
        for d in range(D):
            sl = scr[:, 63 - d : 63 - d + W, d, :]  # [NCHUNK, W, HC(slots)]
            sl = sl[:, :, perm]
            out[b, d, half * HS : (half + 1) * HS, :] = (
                sl.transpose(0, 2, 1).reshape(HS, W)
            )
    out *= 1.0 / C
    return out


def _make_in_maps(x):
    x = np.asarray(x)
    assert x.shape == (B, 2 * C, H, W), x.shape
    in_maps = []
    for core in range(NCORES):
        b, half = core // 2, core % 2
        sh = slice(half * HS, (half + 1) * HS)
        f1 = np.asarray(x[b, :C, sh, :], dtype=NP_IN)  # [C, HS, W]
        f2 = np.asarray(x[b, C:, sh, :], dtype=NP_IN)
        # pack: arr[64*par + c, j, w] = f[c, 2j + par, w]
        f1p = np.empty((128, HS // 2, W), dtype=NP_IN)
        f1p[:C] = f1[:, 0::2, :]
        f1p[C:] = f1[:, 1::2, :]
        f2p = np.zeros((128, HS // 2, WPAD), dtype=NP_IN)
        f2p[:C, :, D : D + W] = f2[:, 0::2, :]
        f2p[C:, :, D : D + W] = f2[:, 1::2, :]
        in_maps.append({"f1s": f1p, "f2s": f2p})
    return in_maps


def kernel(x, max_disparity):
    assert int(max_disparity) == D
    res = _run_on_hw(_make_in_maps(x))
    scr_cores = [res.results[core]["scr"] for core in range(NCORES)]
    return _assemble(scr_cores)


# revision 20
# speedup vs baseline: 1.7426x; 1.1716x over previous
"""Trainium2 Bass kernel for FlowNet/stereo-style horizontal correlation.

Reference semantics (per batch sample b):
    x: [2C, H, W] fp32, f1 = x[:C], f2 = x[C:]
    out[d, h, w] = (1/C) * sum_c f1[c, h, w] * f2[c, h, w - d]   (zero-padded)
with C = 64, D = max_disparity = 64, H = 256, W = 512, B = 4.

Strategy (8 NeuronCores):
  Shard batch (4) x H-halves (2) -> 8 shards of [128c2, 128h, 512w].

  On-device, per (h, t) with t in {0, 64, ..., 448}:
    one TensorE matmul: stationary lhsT = f2 window [c=64, 128 cols]
    covering w' in [t-63, t+64] (zero-padded at edges), moving rhs =
    f1 [c=64, 64 cols] covering w in [t, t+64).  PSUM cell (m, n) =
    sum_c f2[c, t-63+m] * f1[c, t+n] = unnormalized out[d=63+n-m, w=t+n].

  The [128, 64] rectangle is copied (DVE/ACT) into a staging tile and
  DMAed to a DRAM scratch laid out in "band lines": line s~ = t + m,
  slot j^ = 63 + n - m, i.e. flat q = (t+m)*(PITCH*HC) + (63+n-m)*HC + hh.
  Per-partition DRAM offsets are affine in (m, t, n, hh) -> a single
  legal strided DMA with HC-elem-granular contiguous runs of 64*HC
  elems.  Cells with d outside [0, 64) provably land in pad slots
  [64, 128) of some line and are never read.  Host assembles:
  out[d, h, w] = scr[w-d+63, d, h] * (1/C).

  Inputs are packed on the host so each per-chunk DMA covers all 128
  SBUF partitions: partition p = 64*(h&1) + c, free = (h//2, w).
"""

import os
import sys

sys.path.insert(0, "/opt/trn_rl_repo")

import numpy as np

import concourse.bass as bass
import concourse.mybir as mybir
import concourse.tile as tile
from concourse import bacc, bass_utils

# problem constants (hardcoded per contract)
B = 4
C = 64
D = 64
H = 256
W = 512
NCORES = 8
HS = H // 2          # 128 rows per core
HC = 16              # h-chunk size
HP = HC // 2         # h-pairs per chunk (h-parity packed on partitions)
NCHUNK = HS // HC    # 8
TSTEP = 64
NT = W // TSTEP      # 8
PITCH = 128          # j^ pitch (valid slots [0, 64), pad [64, 128))
LINES = W + D        # 576 band lines (s~ = t + m in [0, 575])
WPAD = 584           # f2 padded row: cols [0,64)=0, [64,576)=data, 576.. pad

DT_IN = mybir.dt.float16
DT_OUT = mybir.dt.float16
NP_IN = np.float16
NP_OUT = np.float16


def _corr_kernel(tc, f1_ap, f2_ap, scr_ap):
    nc = tc.nc
    scr_t = scr_ap.tensor
    with (
        tc.tile_pool(name="io", bufs=2) as iopool,
        tc.tile_pool(name="stage", bufs=2) as stpool,
        tc.tile_pool(name="ps", bufs=int(os.environ.get("K_PS_BUFS", "3")), space="PSUM") as pspool,
    ):
        for cc in range(NCHUNK):
            j0 = cc * HP
            f1 = iopool.tile([128, HP, W], DT_IN, tag="f1")
            f2 = iopool.tile([128, HP, WPAD], DT_IN, tag="f2")
            nc.sync.dma_start(f1[:, :, :], f1_ap[:, j0 : j0 + HP, :])
            nc.sync.dma_start(f2[:, :, :], f2_ap[:, j0 : j0 + HP, :])

            st = stpool.tile([128, NT * TSTEP * HC], DT_OUT, tag="st")
            st4 = st.rearrange("p (t n h) -> p t n h", t=NT, n=TSTEP, h=HC)
            ci = 0
            for th in range(NT):
                t = th * TSTEP
                # Two PSUM groups (even-h rows 0-63, odd-h rows 64-127 of the
                # parity-packed input tiles), matmuls interleaved pairwise so
                # the PE runs them on different row groups concurrently and
                # next LDWEIGHTS overlaps the other group's matmul.  Each
                # group's 8 matmuls write one PSUM bank h-interleaved via a
                # strided output AP, so the staging copy reads contiguously.
                ptE = pspool.tile([128, TSTEP, 8], mybir.dt.float32, tag="ptE")
                ptO = pspool.tile([128, TSTEP, 8], mybir.dt.float32, tag="ptO")
                for hj in range(8):
                    for par, pt in ((0, ptE), (1, ptO)):
                        p0 = C * par
                        # lhsT: f2 cols [t-63, t+65) -> padded cols [t+1, t+129)
                        nc.tensor.matmul(
                            pt[:, :, hj],
                            f2[p0 : p0 + C, hj, t + 1 : t + 129],
                            f1[p0 : p0 + C, hj, t : t + TSTEP],
                        )
                act_mod = int(os.environ.get("K_ACT_MOD", "3"))
                for oct_, pt in ((0, ptE), (1, ptO)):
                    dst = st4[:, th, :, oct_ * 8 : (oct_ + 1) * 8]
                    if act_mod and ci % act_mod == act_mod - 1:
                        nc.scalar.copy(dst, pt[:, :, :])
                    else:
                        nc.vector.tensor_copy(dst, pt[:, :, :])
                    ci += 1

            # staging tile -> DRAM scratch (band-line layout), one DMA
            dram_ap = bass.AP(
                tensor=scr_t,
                offset=cc * LINES * PITCH * HC + 63 * HC,
                ap=[
                    [(PITCH - 1) * HC, 128],   # m (partition)
                    [TSTEP * PITCH * HC, NT],  # t^
                    [HC, TSTEP],               # n
                    [1, HC],                   # hh
                ],
            )
            nc.sync.dma_start(dram_ap, st4[:, :, :, :])


def _build():
    nc = bacc.Bacc("TRN2", target_bir_lowering=False, debug=False)
    f1s = nc.dram_tensor("f1s", [128, HS // 2, W], DT_IN, kind="ExternalInput")
    f2s = nc.dram_tensor("f2s", [128, HS // 2, WPAD], DT_IN, kind="ExternalInput")
    scr = nc.dram_tensor(
        "scr", [NCHUNK, LINES, PITCH, HC], DT_OUT, kind="ExternalOutput"
    )
    with tile.TileContext(nc) as tc:
        _corr_kernel(tc, f1s.ap(), f2s.ap(), scr.ap())
    nc.compile()
    return nc


def _run_on_hw(in_maps, trace=False, **kw):
    nc = _build()
    return bass_utils.run_bass_kernel_spmd(
        nc, in_maps, core_ids=list(range(NCORES)), trace=trace, **kw
    )


def _assemble(scr_cores):
    """scr_cores: list of 8 arrays [NCHUNK, LINES, PITCH, HC] -> [B, D, H, W]."""
    out = np.empty((B, D, H, W), dtype=np.float32)
    for core in range(NCORES):
        b, half = core // 2, core % 2
        scr = scr_cores[core].astype(np.float32, copy=False)
        # staging slot s holds h = 2*(s % 8) + (s // 8) within the chunk
        perm = [(h % 2) * 8 + h // 2 for h in range(HC)]
        for d in range(D):
            sl = scr[:, 63 - d : 63 - d + W, d, :]  # [NCHUNK, W, HC(slots)]
            sl = sl[:, :, perm]
            out[b, d, half * HS : (half + 1) * HS, :] = (
                sl.transpose(0, 2, 1).reshape(HS, W)
            )
    out *= 1.0 / C
    return out


def _make_in_maps(x):
    x = np.asarray(x)
    assert x.shape == (B, 2 * C, H, W), x.shape
    in_maps = []
    for core in range(NCORES):
        b, half = core // 2, core % 2
        sh = slice(half * HS, (half + 1) * HS)
        f1 = np.asarray(x[b, :C, sh, :], dtype=NP_IN)  # [C, HS, W]
        f2 = np.asarray(x[b, C:, sh, :], dtype=NP_IN)
        # pack: arr[64*par + c, j, w] = f[c, 2j + par, w]
        f1p = np.empty((128, HS // 2, W), dtype=NP_IN)
        f1p[:C] = f1[:, 0::2, :]
        f1p[C:] = f1[:, 1::2, :]
        f2p = np.zeros((128, HS // 2, WPAD), dtype=NP_IN)
        f2p[:C, :, D : D + W] = f2[:, 0::2, :]
        f2p[C:, :, D : D + W] = f2[:, 1::2, :]
        in_maps.append({"f1s": f1p, "f2s": f2p})
    return in_maps


def kernel(x, max_disparity):
    assert int(max_disparity) == D
    res = _run_on_hw(_make_in_maps(x))
    scr_cores = [res.results[core]["scr"] for core in range(NCORES)]
    return _assemble(scr_cores)
